# revision 13
# baseline (speedup 1.0000x reference)
"""DOTA_mix.fit() streaming EM update on Trainium2 — 8-core K-sharded SPMD.

kernel(x, gamma_class, mu, var, pi, c) -> (mu_new, var_new, c_new, pi_new)

Strategy (per core, classes sharded 1000 -> 8 x 125):
  J[b,km] = log pi + log N(x | mu,var) is computed as ONE fused matmul chain
    J = x2 @ W1c + x @ W2c + constc      (contraction dim 512+512+1)
  where W1c/W2c/constc are per-class mean-centered over the 4 modes, which
  makes mean_m J == 0 exactly, so softmax over modes needs NO max pass
  (centering is softmax-invariant; an extra -40 shift bounds exp above).
  gamma = gamma_class * softmax_m(J);  then one more matmul batch:
    [weighted_x | weighted_x_sq] = gamma^T @ [x | x*x],  sum_gamma = 1^T gamma
  followed by the cheap EM update algebra.
Matmuls run in fp32r (TRN2 fast-fp32, ~13-bit mantissa) at full rate.
"""
import numpy as np
import concourse.bass as bass
import concourse.bacc as bacc
import concourse.tile as tile
from concourse import mybir
from concourse.masks import make_identity
from concourse.bass_utils import run_bass_kernel_spmd

F32 = mybir.dt.float32
F32R = mybir.dt.float32r
AF = mybir.ActivationFunctionType
OP = mybir.AluOpType
AX = mybir.AxisListType

B, K, M, D = 2048, 1000, 4, 512
NCORES = 8
KC = K // NCORES          # 125 classes per core
KM = KC * M               # 500 (class,mode) rows per core
NBT = B // 128            # 16 batch tiles
EPS, TINY, VARMIN = 1e-3, 1e-10, 1e-8
SHIFT = 40.0
CH = [(0, 128), (128, 128), (256, 128), (384, 116)]  # km chunks


def build_nc():
    nc = bacc.Bacc("TRN2", target_bir_lowering=False, debug=False)

    xT_d = nc.dram_tensor("xT", [D, B], F32, kind="ExternalInput")
    x_d = nc.dram_tensor("x", [B, D], F32, kind="ExternalInput")
    varT_d = nc.dram_tensor("varT", [D, KM], F32, kind="ExternalInput")
    muT_d = nc.dram_tensor("muT", [D, KM], F32, kind="ExternalInput")
    var_d = nc.dram_tensor("var", [KM, D], F32, kind="ExternalInput")
    mu_d = nc.dram_tensor("mu", [KM, D], F32, kind="ExternalInput")
    gc_d = nc.dram_tensor("gc", [B, KC], F32, kind="ExternalInput")
    c_d = nc.dram_tensor("c", [1, KM], F32, kind="ExternalInput")
    pi_d = nc.dram_tensor("pi", [1, KM], F32, kind="ExternalInput")

    muo_d = nc.dram_tensor("mu_new", [KM, D], F32, kind="ExternalOutput")
    varo_d = nc.dram_tensor("var_new", [KM, D], F32, kind="ExternalOutput")
    co_d = nc.dram_tensor("c_new", [1, KM], F32, kind="ExternalOutput")
    pio_d = nc.dram_tensor("pi_new", [1, KM], F32, kind="ExternalOutput")

    with tile.TileContext(nc) as tc:
        import contextlib
        with contextlib.ExitStack() as ctx:
            A = ctx.enter_context(tc.tile_pool(name="A", bufs=1))

            # ---- persistent tiles -------------------------------------------------
            gam_sb = A.tile([128, NBT, KM], F32R, tag="gam")    # responsibilities
            ones_col = A.tile([128, 1], F32, tag="ones_col")
            ones_colr = A.tile([128, 1], F32R, tag="ones_colr")
            ones_row = A.tile([1, 128], F32, tag="ones_row")
            ones_rowr = A.tile([1, 128], F32R, tag="ones_rowr")
            constr = A.tile([1, KM], F32R, tag="constr")
            c_sb = A.tile([1, KM], F32, tag="c_sb")
            pi_sb = A.tile([1, KM], F32, tag="pi_sb")

            nc.sync.dma_start(c_sb[:], c_d[:])
            nc.sync.dma_start(pi_sb[:], pi_d[:])
            nc.vector.memset(ones_col[:], 1.0)
            nc.vector.tensor_copy(ones_colr[:], ones_col[:])
            nc.vector.memset(ones_row[:], 1.0)
            nc.vector.tensor_copy(ones_rowr[:], ones_row[:])

            psSG = ctx.enter_context(tc.tile_pool(name="psSG", bufs=1, space="PSUM"))
            sg_ps = None

            with tc.tile_pool(name="BP", bufs=1) as BP:
                xtr_sb = BP.tile([128, 4, B], F32R, tag="xtr")    # d-major x, f32r
                xxT_sb = BP.tile([128, 4, B], F32R, tag="xxT")    # (x*x)^T rounded
                W1r = BP.tile([128, 4, KM], F32R, tag="W1r")      # centered -0.5/var, d-major
                W2r = BP.tile([128, 4, KM], F32R, tag="W2r")      # centered mu/var, d-major

                # ---- phase 0: load + square + round x/xT; W prep -----------------
                with tc.tile_pool(name="CP", bufs=1) as CP, \
                     tc.tile_pool(name="psR", bufs=1, space="PSUM") as psR:
                    for i in range(4):
                        xTf = CP.tile([128, B], F32, tag="xTf", bufs=2)
                        nc.sync.dma_start(
                            xTf[:], xT_d[:].rearrange("(c p) b -> p c b", p=128)[:, i, :])
                        nc.scalar.activation(xxT_sb[:, i, :], xTf[:], AF.Square)
                        nc.gpsimd.tensor_copy(xtr_sb[:, i, :], xTf[:])
                    ld_ps = psR.tile([1, 512], F32, tag="ld")
                    m2_ps = psR.tile([1, 512], F32, tag="m2")

                    for i in range(4):
                        varT = CP.tile([128, KM], F32, tag="varT", bufs=2)
                        muT = CP.tile([128, KM], F32, tag="muT", bufs=2)
                        nc.sync.dma_start(
                            varT[:], varT_d[:].rearrange("(c p) k -> p c k", p=128)[:, i, :])
                        nc.sync.dma_start(
                            muT[:], muT_d[:].rearrange("(c p) k -> p c k", p=128)[:, i, :])
                        inv = CP.tile([128, KM], F32, tag="inv")
                        lg = CP.tile([128, KM], F32, tag="lgw")
                        gmean = CP.tile([128, KC], F32, tag="gmean")

                        cv = varT[:]
                        nc.vector.tensor_scalar_add(cv, cv, EPS)   # in place
                        nc.vector.reciprocal(inv[:], cv)
                        nc.scalar.activation(lg[:], cv, AF.Ln)
                        # reduction over d via ones-matmul (full fp32 for accuracy)
                        nc.tensor.matmul(ld_ps[:, :KM], ones_col[:], lg[:],
                                         start=(i == 0), stop=(i == 3))
                        # center W1 = -0.5*inv over modes:  W1c = -0.5*inv + 0.125*gsum(inv)
                        nc.vector.reduce_sum(out=gmean[:],
                                             in_=inv[:].rearrange("p (k m) -> p k m", m=4),
                                             axis=AX.X)
                        nc.vector.tensor_scalar_mul(gmean[:], gmean[:], 0.125)
                        nc.vector.tensor_scalar_mul(
                            W1r[:, i, :].rearrange("p (k m) -> p k m", m=4),
                            inv[:].rearrange("p (k m) -> p k m", m=4), -0.5)
                        nc.gpsimd.tensor_tensor(
                            out=W1r[:, i, :].rearrange("p (k m) -> p k m", m=4),
                            in0=W1r[:, i, :].rearrange("p (k m) -> p k m", m=4),
                            in1=gmean[:].unsqueeze(-1).broadcast_to((128, KC, 4)),
                            op=OP.add)
                        # w2f reuses lg's slot (after the ld matmul consumed lg)
                        w2f = CP.tile([128, KM], F32, tag="lgw")
                        nc.gpsimd.tensor_mul(w2f[:], muT[:], inv[:])
                        # center W2 = mu*inv over modes
                        nc.vector.reduce_sum(out=gmean[:],
                                             in_=w2f[:].rearrange("p (k m) -> p k m", m=4),
                                             axis=AX.X)
                        nc.vector.tensor_scalar_mul(gmean[:], gmean[:], 0.25)
                        nc.vector.tensor_tensor(
                            out=W2r[:, i, :].rearrange("p (k m) -> p k m", m=4),
                            in0=w2f[:].rearrange("p (k m) -> p k m", m=4),
                            in1=gmean[:].unsqueeze(-1).broadcast_to((128, KC, 4)),
                            op=OP.subtract)
                        # mwi = mu^2/var reuses inv's slot (inv fully consumed)
                        mwi = CP.tile([128, KM], F32, tag="inv")
                        nc.gpsimd.tensor_mul(mwi[:], muT[:], w2f[:])
                        nc.tensor.matmul(m2_ps[:, :KM], ones_col[:], mwi[:],
                                         start=(i == 0), stop=(i == 3))

                    # const row: log(pi+TINY) - 0.5*(log_det+m2), centered, -SHIFT
                    t_row = CP.tile([1, KM], F32, tag="t_row")
                    pl_row = CP.tile([1, KM], F32, tag="pl_row")
                    g_row = CP.tile([1, KC], F32, tag="g_row")
                    nc.vector.tensor_scalar_add(pl_row[:], pi_sb[:], TINY)
                    nc.scalar.activation(pl_row[:], pl_row[:], AF.Ln)
                    nc.scalar.copy(t_row[:], m2_ps[:, :KM])
                    nc.vector.tensor_tensor(out=t_row[:], in0=ld_ps[:, :KM],
                                            in1=t_row[:], op=OP.add)
                    nc.vector.tensor_scalar(out=t_row[:], in0=t_row[:],
                                            scalar1=-0.5, scalar2=None, op0=OP.mult)
                    nc.vector.tensor_tensor(out=t_row[:], in0=t_row[:], in1=pl_row[:],
                                            op=OP.add)
                    nc.vector.reduce_sum(out=g_row[:],
                                         in_=t_row[:].rearrange("p (k m) -> p k m", m=4),
                                         axis=AX.X)
                    nc.vector.tensor_scalar(out=g_row[:], in0=g_row[:], scalar1=0.25,
                                            scalar2=SHIFT, op0=OP.mult, op1=OP.add)
                    nc.vector.tensor_tensor(
                        out=constr[:].rearrange("p (k m) -> p k m", m=4),
                        in0=t_row[:].rearrange("p (k m) -> p k m", m=4),
                        in1=g_row[:].unsqueeze(-1).broadcast_to((1, KC, 4)),
                        op=OP.subtract)

                # ---- phase 1: J matmul + modewise softmax + gamma ----------------
                sg_ps = psSG.tile([1, 512], F32, tag="sg")
                with tc.tile_pool(name="GP", bufs=1) as GP, \
                     tc.tile_pool(name="DP", bufs=3) as DP, \
                     tc.tile_pool(name="psJ", bufs=2, space="PSUM") as psJ:
                    gc_sb = GP.tile([128, NBT, KC], F32, tag="gc")
                    nc.sync.dma_start(gc_sb[:], gc_d[:].rearrange("(n p) k -> p n k", p=128))
                    for bt in range(NBT):
                        bsl = slice(bt * 128, (bt + 1) * 128)
                        J_ps = psJ.tile([128, 512], F32, tag="J")
                        for i in range(4):
                            nc.tensor.matmul(J_ps[:, :KM], xxT_sb[:, i, bsl],
                                             W1r[:, i, :], start=(i == 0), stop=False)
                        for i in range(4):
                            nc.tensor.matmul(J_ps[:, :KM],
                                             xtr_sb[:, i, bsl],
                                             W2r[:, i, :], start=False, stop=False)
                        nc.tensor.matmul(J_ps[:, :KM], ones_rowr[:], constr[:],
                                         start=False, stop=True)
                        E = DP.tile([128, KM], F32, tag="E")
                        nc.scalar.activation(E[:], J_ps[:, :KM], AF.Exp)
                        s4 = DP.tile([128, KC], F32, tag="s4")
                        nc.vector.reduce_sum(out=s4[:],
                                             in_=E[:].rearrange("p (k m) -> p k m", m=4),
                                             axis=AX.X)
                        wgt = DP.tile([128, KC], F32, tag="wgt")
                        nc.vector.reciprocal(wgt[:], s4[:])
                        nc.vector.tensor_mul(wgt[:], wgt[:], gc_sb[:, bt, :])
                        nc.gpsimd.tensor_tensor(
                            out=gam_sb[:, bt, :].rearrange("p (k m) -> p k m", m=4),
                            in0=E[:].rearrange("p (k m) -> p k m", m=4),
                            in1=wgt[:].unsqueeze(-1).broadcast_to((128, KC, 4)),
                            op=OP.mult)
                        nc.tensor.matmul(sg_ps[:, :KM], ones_colr[:], gam_sb[:, bt, :],
                                         start=(bt == 0), stop=(bt == NBT - 1))

            # ---- phase 2a: per-km scalars (row layout), c_new, pi_new ------------
            F2 = ctx.enter_context(tc.tile_pool(name="F2", bufs=1))
            cn_row = F2.tile([1, KM], F32, tag="cn_row")
            pin_row = F2.tile([1, KM], F32, tag="pin_row")
            scals = F2.tile([5, KM], F32, tag="scals")           # r, a1, a2, a3, sqrt(a3) rows
            cols = F2.tile([128, 4, 5], F32, tag="cols")         # transposed scalars / km chunk
            pig = F2.tile([1, KC], F32, tag="pig")
            ident4 = F2.tile([5, 5], F32, tag="ident4")
            make_identity(nc, ident4[:])

            r_row = F2.tile([1, KM], F32, tag="r_row")
            a_row = F2.tile([1, KM], F32, tag="a_row")
            nc.vector.tensor_tensor(out=cn_row[:], in0=c_sb[:], in1=sg_ps[:, :KM],
                                    op=OP.add)
            nc.sync.dma_start(co_d[:], cn_row[:])
            # r = 1/(c_new + TINY)
            nc.vector.tensor_scalar_add(r_row[:], cn_row[:], TINY)
            nc.vector.reciprocal(r_row[:], r_row[:])
            nc.sync.dma_start(scals[0:1, :], r_row[:])
            # a1 = c * r
            nc.vector.tensor_tensor(out=a_row[:], in0=c_sb[:], in1=r_row[:],
                                    op=OP.mult)
            nc.sync.dma_start(scals[1:2, :], a_row[:])
            # a2 = -2r
            a2_row = F2.tile([1, KM], F32, tag="a2_row")
            nc.vector.tensor_scalar_mul(a2_row[:], r_row[:], -2.0)
            nc.sync.dma_start(scals[2:3, :], a2_row[:])
            # a3 = sum_gamma * r
            a3_row = F2.tile([1, KM], F32, tag="a3_row")
            nc.vector.tensor_tensor(out=a3_row[:], in0=sg_ps[:, :KM],
                                    in1=r_row[:], op=OP.mult)
            nc.sync.dma_start(scals[3:4, :], a3_row[:])
            a3s_row = F2.tile([1, KM], F32, tag="a3s_row")
            nc.scalar.activation(a3s_row[:], a3_row[:], AF.Sqrt)
            nc.sync.dma_start(scals[4:5, :], a3s_row[:])
            # pi_new = c_new / (gsum_m c_new + TINY)
            nc.vector.reduce_sum(out=pig[:],
                                 in_=cn_row[:].rearrange("p (k m) -> p k m", m=4),
                                 axis=AX.X)
            nc.vector.tensor_scalar_add(pig[:], pig[:], TINY)
            nc.vector.reciprocal(pig[:], pig[:])
            nc.vector.tensor_tensor(
                out=pin_row[:].rearrange("p (k m) -> p k m", m=4),
                in0=cn_row[:].rearrange("p (k m) -> p k m", m=4),
                in1=pig[:].unsqueeze(-1).broadcast_to((1, KC, 4)), op=OP.mult)
            nc.sync.dma_start(pio_d[:], pin_row[:])

            # transpose scalar rows into per-chunk columns
            with tc.tile_pool(name="psT", bufs=2, space="PSUM") as psT:
                for j, (j0, cw) in enumerate(CH):
                    t_ps = psT.tile([128, 5], F32, tag="tps")
                    nc.tensor.transpose(t_ps[:cw, :], scals[:, j0:j0 + cw],
                                        ident4[:])
                    nc.scalar.copy(cols[:cw, j, :], t_ps[:cw, :])

            # ---- phase 2b: gamma^T @ [x|xx] and EM update ------------------------
            with tc.tile_pool(name="EP", bufs=2) as EP, \
                 tc.tile_pool(name="psW", bufs=2, space="PSUM") as psW:
                xr_sb = EP.tile([128, NBT, D], F32R, tag="xr", bufs=1)
                xx_sb = EP.tile([128, NBT, D], F32R, tag="xx", bufs=1)
                for i in range(4):
                    xf = EP.tile([128, 4, D], F32, tag="xf")
                    nc.sync.dma_start(
                        xf[:], x_d[:].rearrange("(n p) d -> p n d", p=128)[:, 4 * i:4 * i + 4, :])
                    nc.scalar.activation(xx_sb[:, 4 * i:4 * i + 4, :], xf[:], AF.Square)
                    nc.gpsimd.tensor_copy(xr_sb[:, 4 * i:4 * i + 4, :], xf[:])
                for j, (j0, cw) in enumerate(CH):
                    ksl = slice(j0, j0 + cw)
                    wx_ps = psW.tile([128, 512], F32, tag="wx")
                    wq_ps = psW.tile([128, 512], F32, tag="wq")
                    for bt in range(NBT):
                        nc.tensor.matmul(wx_ps[:cw, :], gam_sb[:, bt, ksl],
                                         xr_sb[:, bt, :],
                                         start=(bt == 0), stop=(bt == NBT - 1))
                    for bt in range(NBT):
                        nc.tensor.matmul(wq_ps[:cw, :], gam_sb[:, bt, ksl],
                                         xx_sb[:, bt, :],
                                         start=(bt == 0), stop=(bt == NBT - 1))

                    r_c = cols[:cw, j, 0:1]
                    a1_c = cols[:cw, j, 1:2]
                    a2_c = cols[:cw, j, 2:3]
                    a3s_c = cols[:cw, j, 4:5]

                    mu_t = EP.tile([128, D], F32, tag="mu_t")
                    var_t = EP.tile([128, D], F32, tag="var_t")
                    nc.sync.dma_start(mu_t[:cw, :], mu_d[ksl, :])
                    nc.sync.dma_start(var_t[:cw, :], var_d[ksl, :])

                    # mu_new = mu*a1 + wx*r
                    u1 = EP.tile([128, D], F32, tag="u1")
                    u2 = EP.tile([128, D], F32, tag="u2")
                    nc.scalar.activation(u1[:cw, :], wx_ps[:cw, :], AF.Copy, scale=r_c)
                    nc.scalar.activation(u2[:cw, :], mu_t[:cw, :], AF.Copy, scale=a1_c)
                    muo_t = EP.tile([128, D], F32, tag="muo_t")
                    nc.gpsimd.tensor_tensor(out=muo_t[:cw, :], in0=u1[:cw, :],
                                            in1=u2[:cw, :], op=OP.add)
                    nc.sync.dma_start(muo_d[ksl, :], muo_t[:cw, :])

                    # var_new = max(wxsq*r + var*a1 + (mu*a2)*wx + (mu*sqrt(a3))^2, VARMIN)
                    ma2 = EP.tile([128, D], F32, tag="ma2")
                    nc.scalar.activation(ma2[:cw, :], mu_t[:cw, :], AF.Copy, scale=a2_c)
                    q = EP.tile([128, D], F32, tag="q")
                    nc.vector.tensor_mul(q[:cw, :], ma2[:cw, :], wx_ps[:cw, :])
                    m2e = EP.tile([128, D], F32, tag="m2e")
                    nc.scalar.activation(m2e[:cw, :], mu_t[:cw, :], AF.Square,
                                         scale=a3s_c)
                    v1 = EP.tile([128, D], F32, tag="v1")
                    v2 = EP.tile([128, D], F32, tag="v2")
                    nc.scalar.activation(v1[:cw, :], wq_ps[:cw, :], AF.Copy, scale=r_c)
                    nc.scalar.activation(v2[:cw, :], var_t[:cw, :], AF.Copy, scale=a1_c)
                    nc.gpsimd.tensor_tensor(out=v1[:cw, :], in0=v1[:cw, :],
                                            in1=v2[:cw, :], op=OP.add)
                    nc.vector.tensor_tensor(out=q[:cw, :], in0=q[:cw, :],
                                            in1=m2e[:cw, :], op=OP.add)
                    varo_t = EP.tile([128, D], F32, tag="varo_t")
                    nc.vector.tensor_tensor(out=varo_t[:cw, :], in0=v1[:cw, :],
                                            in1=q[:cw, :], op=OP.add)
                    nc.vector.tensor_scalar_max(varo_t[:cw, :], varo_t[:cw, :], VARMIN)
                    nc.sync.dma_start(varo_d[ksl, :], varo_t[:cw, :])

    nc.compile()
    return nc


_NC = None


def _get_nc():
    global _NC
    if _NC is None:
        _NC = build_nc()
    return _NC


def make_in_maps(x, gamma_class, mu, var, pi, c):
    x = np.ascontiguousarray(np.asarray(x, np.float32))
    gamma_class = np.ascontiguousarray(np.asarray(gamma_class, np.float32))
    mu = np.asarray(mu, np.float32)
    var = np.asarray(var, np.float32)
    pi = np.asarray(pi, np.float32)
    c = np.asarray(c, np.float32)
    xT = np.ascontiguousarray(x.T)
    in_maps = []
    for i in range(NCORES):
        ks, ke = i * KC, (i + 1) * KC
        mu_i = np.ascontiguousarray(mu[ks:ke].reshape(KM, D))
        var_i = np.ascontiguousarray(var[ks:ke].reshape(KM, D))
        in_maps.append({
            "xT": xT,
            "x": x,
            "varT": np.ascontiguousarray(var_i.T),
            "muT": np.ascontiguousarray(mu_i.T),
            "var": var_i,
            "mu": mu_i,
            "gc": np.ascontiguousarray(gamma_class[:, ks:ke]),
            "c": np.ascontiguousarray(c[ks:ke].reshape(1, KM)),
            "pi": np.ascontiguousarray(pi[ks:ke].reshape(1, KM)),
        })
    return in_maps


def kernel(x, gamma_class, mu, var, pi, c):
    in_maps = make_in_maps(x, gamma_class, mu, var, pi, c)
    nc = _get_nc()
    out = run_bass_kernel_spmd(nc, in_maps, core_ids=list(range(NCORES)))
    rs = out.results
    mu_new = np.concatenate([rs[i]["mu_new"].reshape(KC, M, D) for i in range(NCORES)], 0)
    var_new = np.concatenate([rs[i]["var_new"].reshape(KC, M, D) for i in range(NCORES)], 0)
    c_new = np.concatenate([rs[i]["c_new"].reshape(KC, M) for i in range(NCORES)], 0)
    pi_new = np.concatenate([rs[i]["pi_new"].reshape(KC, M) for i in range(NCORES)], 0)
    return (np.asarray(mu_new, np.float32), np.asarray(var_new, np.float32),
            np.asarray(c_new, np.float32), np.asarray(pi_new, np.float32))


# revision 16
# speedup vs baseline: 1.2086x; 1.2086x over previous
"""DOTA_mix.fit() streaming EM update on Trainium2 — 8-core K-sharded SPMD.

kernel(x, gamma_class, mu, var, pi, c) -> (mu_new, var_new, c_new, pi_new)

Strategy (per core, classes sharded 1000 -> 8 x 125):
  J[b,km] = log pi + log N(x | mu,var) is computed as ONE fused matmul chain
    J = x2 @ W1c + x @ W2c + constc      (contraction dim 512+512+1)
  where W1c/W2c/constc are per-class mean-centered over the 4 modes, which
  makes mean_m J == 0 exactly, so softmax over modes needs NO max pass
  (centering is softmax-invariant; an extra -40 shift bounds exp above).
  gamma = gamma_class * softmax_m(J);  then one more matmul batch:
    [weighted_x | weighted_x_sq] = gamma^T @ [x | x*x],  sum_gamma = 1^T gamma
  followed by the cheap EM update algebra.
Matmuls run in fp32r (TRN2 fast-fp32, ~13-bit mantissa) at full rate.
"""
import numpy as np
import concourse.bass as bass
import concourse.bacc as bacc
import concourse.tile as tile
from concourse import mybir
from concourse.masks import make_identity
from concourse.bass_utils import run_bass_kernel_spmd

F32 = mybir.dt.float32
F32R = mybir.dt.float32r
AF = mybir.ActivationFunctionType
OP = mybir.AluOpType
AX = mybir.AxisListType

B, K, M, D = 2048, 1000, 4, 512
NCORES = 8
KC = K // NCORES          # 125 classes per core
KM = KC * M               # 500 (class,mode) rows per core
NBT = B // 128            # 16 batch tiles
EPS, TINY, VARMIN = 1e-3, 1e-10, 1e-8
SHIFT = 40.0
CH = [(0, 128), (128, 128), (256, 128), (384, 116)]  # km chunks


def build_nc():
    nc = bacc.Bacc("TRN2", target_bir_lowering=False, debug=False)

    xT_d = nc.dram_tensor("xT", [D, B], F32, kind="ExternalInput")
    x_d = nc.dram_tensor("x", [B, D], F32, kind="ExternalInput")
    varT_d = nc.dram_tensor("varT", [D, KM], F32, kind="ExternalInput")
    muT_d = nc.dram_tensor("muT", [D, KM], F32, kind="ExternalInput")
    var_d = nc.dram_tensor("var", [KM, D], F32, kind="ExternalInput")
    mu_d = nc.dram_tensor("mu", [KM, D], F32, kind="ExternalInput")
    gc_d = nc.dram_tensor("gc", [B, KC], F32, kind="ExternalInput")
    c_d = nc.dram_tensor("c", [1, KM], F32, kind="ExternalInput")
    pi_d = nc.dram_tensor("pi", [1, KM], F32, kind="ExternalInput")

    muo_d = nc.dram_tensor("mu_new", [KM, D], F32, kind="ExternalOutput")
    varo_d = nc.dram_tensor("var_new", [KM, D], F32, kind="ExternalOutput")
    co_d = nc.dram_tensor("c_new", [1, KM], F32, kind="ExternalOutput")
    pio_d = nc.dram_tensor("pi_new", [1, KM], F32, kind="ExternalOutput")

    with tile.TileContext(nc) as tc:
        import contextlib
        with contextlib.ExitStack() as ctx:
            A = ctx.enter_context(tc.tile_pool(name="A", bufs=1))

            # ---- persistent tiles -------------------------------------------------
            gam_sb = A.tile([128, NBT, KM], F32R, tag="gam")    # responsibilities
            ones_col = A.tile([128, 1], F32, tag="ones_col")
            ones_colr = A.tile([128, 1], F32R, tag="ones_colr")
            ones_row = A.tile([1, 128], F32, tag="ones_row")
            ones_rowr = A.tile([1, 128], F32R, tag="ones_rowr")
            constr = A.tile([1, KM], F32R, tag="constr")
            c_sb = A.tile([1, KM], F32, tag="c_sb")
            pi_sb = A.tile([1, KM], F32, tag="pi_sb")

            nc.sync.dma_start(c_sb[:], c_d[:])
            nc.sync.dma_start(pi_sb[:], pi_d[:])
            nc.vector.memset(ones_col[:], 1.0)
            nc.vector.tensor_copy(ones_colr[:], ones_col[:])
            nc.vector.memset(ones_row[:], 1.0)
            nc.vector.tensor_copy(ones_rowr[:], ones_row[:])

            psSG = ctx.enter_context(tc.tile_pool(name="psSG", bufs=1, space="PSUM"))
            sg_ps = None

            with tc.tile_pool(name="BP", bufs=1) as BP:
                xtr_sb = BP.tile([128, 4, B], F32R, tag="xtr")    # d-major x, f32r
                xxT_sb = BP.tile([128, 4, B], F32R, tag="xxT")    # (x*x)^T rounded
                W1r = BP.tile([128, 4, KM], F32R, tag="W1r")      # centered -0.5/var, d-major
                W2r = BP.tile([128, 4, KM], F32R, tag="W2r")      # centered mu/var, d-major

                # ---- phase 0: load + square + round x/xT; W prep -----------------
                with tc.tile_pool(name="CP", bufs=1) as CP, \
                     tc.tile_pool(name="psR", bufs=1, space="PSUM") as psR:
                    for i in range(4):
                        xTf = CP.tile([128, B], F32, tag="xTf", bufs=2)
                        nc.sync.dma_start(
                            xTf[:], xT_d[:].rearrange("(c p) b -> p c b", p=128)[:, i, :])
                        nc.scalar.activation(xxT_sb[:, i, :], xTf[:], AF.Square)
                        nc.scalar.copy(xtr_sb[:, i, :], xTf[:])
                    ld_ps = psR.tile([1, 512], F32, tag="ld")
                    m2_ps = psR.tile([1, 512], F32, tag="m2")

                    for i in range(4):
                        varT = CP.tile([128, KM], F32, tag="varT", bufs=2)
                        muT = CP.tile([128, KM], F32, tag="muT", bufs=2)
                        nc.sync.dma_start(
                            varT[:], varT_d[:].rearrange("(c p) k -> p c k", p=128)[:, i, :])
                        nc.sync.dma_start(
                            muT[:], muT_d[:].rearrange("(c p) k -> p c k", p=128)[:, i, :])
                        inv = CP.tile([128, KM], F32, tag="inv")
                        lg = CP.tile([128, KM], F32, tag="lgw")
                        gmean = CP.tile([128, KC], F32, tag="gmean")

                        cv = varT[:]
                        nc.vector.tensor_scalar_add(cv, cv, EPS)   # in place
                        nc.vector.reciprocal(inv[:], cv)
                        nc.scalar.activation(lg[:], cv, AF.Ln)
                        # reduction over d via ones-matmul (full fp32 for accuracy)
                        nc.tensor.matmul(ld_ps[:, :KM], ones_col[:], lg[:],
                                         start=(i == 0), stop=(i == 3))
                        # center W1 = -0.5*inv over modes:  W1c = -0.5*inv + 0.125*gsum(inv)
                        nc.vector.reduce_sum(out=gmean[:],
                                             in_=inv[:].rearrange("p (k m) -> p k m", m=4),
                                             axis=AX.X)
                        nc.vector.tensor_scalar_mul(gmean[:], gmean[:], 0.125)
                        nc.vector.tensor_scalar_mul(
                            W1r[:, i, :].rearrange("p (k m) -> p k m", m=4),
                            inv[:].rearrange("p (k m) -> p k m", m=4), -0.5)
                        nc.vector.tensor_tensor(
                            out=W1r[:, i, :].rearrange("p (k m) -> p k m", m=4),
                            in0=W1r[:, i, :].rearrange("p (k m) -> p k m", m=4),
                            in1=gmean[:].unsqueeze(-1).broadcast_to((128, KC, 4)),
                            op=OP.add)
                        # w2f reuses lg's slot (after the ld matmul consumed lg)
                        w2f = CP.tile([128, KM], F32, tag="lgw")
                        nc.gpsimd.tensor_mul(w2f[:], muT[:], inv[:])
                        # center W2 = mu*inv over modes
                        nc.vector.reduce_sum(out=gmean[:],
                                             in_=w2f[:].rearrange("p (k m) -> p k m", m=4),
                                             axis=AX.X)
                        nc.vector.tensor_scalar_mul(gmean[:], gmean[:], 0.25)
                        nc.vector.tensor_tensor(
                            out=W2r[:, i, :].rearrange("p (k m) -> p k m", m=4),
                            in0=w2f[:].rearrange("p (k m) -> p k m", m=4),
                            in1=gmean[:].unsqueeze(-1).broadcast_to((128, KC, 4)),
                            op=OP.subtract)
                        # mwi = mu^2/var reuses inv's slot (inv fully consumed)
                        mwi = CP.tile([128, KM], F32, tag="inv")
                        nc.gpsimd.tensor_mul(mwi[:], muT[:], w2f[:])
                        nc.tensor.matmul(m2_ps[:, :KM], ones_col[:], mwi[:],
                                         start=(i == 0), stop=(i == 3))

                    # const row: log(pi+TINY) - 0.5*(log_det+m2), centered, -SHIFT
                    t_row = CP.tile([1, KM], F32, tag="t_row")
                    pl_row = CP.tile([1, KM], F32, tag="pl_row")
                    g_row = CP.tile([1, KC], F32, tag="g_row")
                    nc.vector.tensor_scalar_add(pl_row[:], pi_sb[:], TINY)
                    nc.scalar.activation(pl_row[:], pl_row[:], AF.Ln)
                    nc.scalar.copy(t_row[:], m2_ps[:, :KM])
                    nc.vector.tensor_tensor(out=t_row[:], in0=ld_ps[:, :KM],
                                            in1=t_row[:], op=OP.add)
                    nc.vector.tensor_scalar(out=t_row[:], in0=t_row[:],
                                            scalar1=-0.5, scalar2=None, op0=OP.mult)
                    nc.vector.tensor_tensor(out=t_row[:], in0=t_row[:], in1=pl_row[:],
                                            op=OP.add)
                    nc.vector.reduce_sum(out=g_row[:],
                                         in_=t_row[:].rearrange("p (k m) -> p k m", m=4),
                                         axis=AX.X)
                    nc.vector.tensor_scalar(out=g_row[:], in0=g_row[:], scalar1=0.25,
                                            scalar2=SHIFT, op0=OP.mult, op1=OP.add)
                    nc.vector.tensor_tensor(
                        out=constr[:].rearrange("p (k m) -> p k m", m=4),
                        in0=t_row[:].rearrange("p (k m) -> p k m", m=4),
                        in1=g_row[:].unsqueeze(-1).broadcast_to((1, KC, 4)),
                        op=OP.subtract)

                # ---- phase 1: J matmul + modewise softmax + gamma ----------------
                sg_ps = psSG.tile([1, 512], F32, tag="sg")
                with tc.tile_pool(name="GP", bufs=1) as GP, \
                     tc.tile_pool(name="DP", bufs=3) as DP, \
                     tc.tile_pool(name="psJ", bufs=3, space="PSUM") as psJ:
                    gc_sb = GP.tile([128, NBT, KC], F32, tag="gc")
                    nc.sync.dma_start(gc_sb[:], gc_d[:].rearrange("(n p) k -> p n k", p=128))
                    for bt in range(NBT):
                        bsl = slice(bt * 128, (bt + 1) * 128)
                        J_ps = psJ.tile([128, 512], F32, tag="J")
                        for i in range(4):
                            nc.tensor.matmul(J_ps[:, :KM], xxT_sb[:, i, bsl],
                                             W1r[:, i, :], start=(i == 0), stop=False)
                        for i in range(4):
                            nc.tensor.matmul(J_ps[:, :KM],
                                             xtr_sb[:, i, bsl],
                                             W2r[:, i, :], start=False, stop=False)
                        nc.tensor.matmul(J_ps[:, :KM], ones_rowr[:], constr[:],
                                         start=False, stop=True)
                        E = DP.tile([128, KM], F32, tag="E")
                        nc.scalar.activation(E[:], J_ps[:, :KM], AF.Exp)
                        s4 = DP.tile([128, KC], F32, tag="s4")
                        nc.vector.reduce_sum(out=s4[:],
                                             in_=E[:].rearrange("p (k m) -> p k m", m=4),
                                             axis=AX.X)
                        wgt = DP.tile([128, KC], F32, tag="wgt")
                        nc.vector.reciprocal(wgt[:], s4[:])
                        nc.vector.tensor_mul(wgt[:], wgt[:], gc_sb[:, bt, :])
                        nc.gpsimd.tensor_tensor(
                            out=gam_sb[:, bt, :].rearrange("p (k m) -> p k m", m=4),
                            in0=E[:].rearrange("p (k m) -> p k m", m=4),
                            in1=wgt[:].unsqueeze(-1).broadcast_to((128, KC, 4)),
                            op=OP.mult)
                        nc.tensor.matmul(sg_ps[:, :KM], ones_colr[:], gam_sb[:, bt, :],
                                         start=(bt == 0), stop=(bt == NBT - 1))

            # ---- phase 2a: per-km scalars (row layout), c_new, pi_new ------------
            F2 = ctx.enter_context(tc.tile_pool(name="F2", bufs=1))
            cn_row = F2.tile([1, KM], F32, tag="cn_row")
            pin_row = F2.tile([1, KM], F32, tag="pin_row")
            scals = F2.tile([5, KM], F32, tag="scals")           # r, a1, a2, a3, sqrt(a3) rows
            cols = F2.tile([128, 4, 5], F32, tag="cols")         # transposed scalars / km chunk
            pig = F2.tile([1, KC], F32, tag="pig")
            ident4 = F2.tile([5, 5], F32, tag="ident4")
            make_identity(nc, ident4[:])

            r_row = F2.tile([1, KM], F32, tag="r_row")
            a_row = F2.tile([1, KM], F32, tag="a_row")
            nc.vector.tensor_tensor(out=cn_row[:], in0=c_sb[:], in1=sg_ps[:, :KM],
                                    op=OP.add)
            nc.sync.dma_start(co_d[:], cn_row[:])
            # r = 1/(c_new + TINY)
            nc.vector.tensor_scalar_add(r_row[:], cn_row[:], TINY)
            nc.vector.reciprocal(r_row[:], r_row[:])
            nc.sync.dma_start(scals[0:1, :], r_row[:])
            # a1 = c * r
            nc.vector.tensor_tensor(out=a_row[:], in0=c_sb[:], in1=r_row[:],
                                    op=OP.mult)
            nc.sync.dma_start(scals[1:2, :], a_row[:])
            # a2 = -2r
            a2_row = F2.tile([1, KM], F32, tag="a2_row")
            nc.vector.tensor_scalar_mul(a2_row[:], r_row[:], -2.0)
            nc.sync.dma_start(scals[2:3, :], a2_row[:])
            # a3 = sum_gamma * r
            a3_row = F2.tile([1, KM], F32, tag="a3_row")
            nc.vector.tensor_tensor(out=a3_row[:], in0=sg_ps[:, :KM],
                                    in1=r_row[:], op=OP.mult)
            nc.sync.dma_start(scals[3:4, :], a3_row[:])
            a3s_row = F2.tile([1, KM], F32, tag="a3s_row")
            nc.scalar.activation(a3s_row[:], a3_row[:], AF.Sqrt)
            nc.sync.dma_start(scals[4:5, :], a3s_row[:])
            # pi_new = c_new / (gsum_m c_new + TINY)
            nc.vector.reduce_sum(out=pig[:],
                                 in_=cn_row[:].rearrange("p (k m) -> p k m", m=4),
                                 axis=AX.X)
            nc.vector.tensor_scalar_add(pig[:], pig[:], TINY)
            nc.vector.reciprocal(pig[:], pig[:])
            nc.vector.tensor_tensor(
                out=pin_row[:].rearrange("p (k m) -> p k m", m=4),
                in0=cn_row[:].rearrange("p (k m) -> p k m", m=4),
                in1=pig[:].unsqueeze(-1).broadcast_to((1, KC, 4)), op=OP.mult)
            nc.sync.dma_start(pio_d[:], pin_row[:])

            # transpose scalar rows into per-chunk columns
            with tc.tile_pool(name="psT", bufs=2, space="PSUM") as psT:
                for j, (j0, cw) in enumerate(CH):
                    t_ps = psT.tile([128, 5], F32, tag="tps")
                    nc.tensor.transpose(t_ps[:cw, :], scals[:, j0:j0 + cw],
                                        ident4[:])
                    nc.scalar.copy(cols[:cw, j, :], t_ps[:cw, :])

            # ---- phase 2b: gamma^T @ [x|xx] and EM update ------------------------
            with tc.tile_pool(name="EP", bufs=2) as EP, \
                 tc.tile_pool(name="psW", bufs=2, space="PSUM") as psW:
                xr_sb = EP.tile([128, NBT, D], F32R, tag="xr", bufs=1)
                xx_sb = EP.tile([128, NBT, D], F32R, tag="xx", bufs=1)
                for i in range(4):
                    xf = EP.tile([128, 4, D], F32, tag="xf")
                    nc.sync.dma_start(
                        xf[:], x_d[:].rearrange("(n p) d -> p n d", p=128)[:, 4 * i:4 * i + 4, :])
                    nc.scalar.activation(xx_sb[:, 4 * i:4 * i + 4, :], xf[:], AF.Square)
                    nc.vector.tensor_copy(xr_sb[:, 4 * i:4 * i + 4, :], xf[:])
                for j, (j0, cw) in enumerate(CH):
                    ksl = slice(j0, j0 + cw)
                    wx_ps = psW.tile([128, 512], F32, tag="wx")
                    wq_ps = psW.tile([128, 512], F32, tag="wq")
                    for bt in range(NBT):
                        nc.tensor.matmul(wx_ps[:cw, :], gam_sb[:, bt, ksl],
                                         xr_sb[:, bt, :],
                                         start=(bt == 0), stop=(bt == NBT - 1))
                    for bt in range(NBT):
                        nc.tensor.matmul(wq_ps[:cw, :], gam_sb[:, bt, ksl],
                                         xx_sb[:, bt, :],
                                         start=(bt == 0), stop=(bt == NBT - 1))

                    r_c = cols[:cw, j, 0:1]
                    a1_c = cols[:cw, j, 1:2]
                    a2_c = cols[:cw, j, 2:3]
                    a3s_c = cols[:cw, j, 4:5]

                    mu_t = EP.tile([128, D], F32, tag="mu_t")
                    var_t = EP.tile([128, D], F32, tag="var_t")
                    nc.sync.dma_start(mu_t[:cw, :], mu_d[ksl, :])
                    nc.sync.dma_start(var_t[:cw, :], var_d[ksl, :])

                    # mu_new = mu*a1 + wx*r
                    u1 = EP.tile([128, D], F32, tag="u1")
                    u2 = EP.tile([128, D], F32, tag="u2")
                    nc.vector.tensor_scalar_mul(u1[:cw, :], wx_ps[:cw, :], r_c)
                    nc.scalar.activation(u2[:cw, :], mu_t[:cw, :], AF.Copy, scale=a1_c)
                    muo_t = EP.tile([128, D], F32, tag="muo_t")
                    nc.gpsimd.tensor_tensor(out=muo_t[:cw, :], in0=u1[:cw, :],
                                            in1=u2[:cw, :], op=OP.add)
                    nc.sync.dma_start(muo_d[ksl, :], muo_t[:cw, :])

                    # var_new = max(wxsq*r + var*a1 + (mu*a2)*wx + (mu*sqrt(a3))^2, VARMIN)
                    ma2 = EP.tile([128, D], F32, tag="ma2")
                    nc.scalar.activation(ma2[:cw, :], mu_t[:cw, :], AF.Copy, scale=a2_c)
                    q = EP.tile([128, D], F32, tag="q")
                    nc.vector.tensor_mul(q[:cw, :], ma2[:cw, :], wx_ps[:cw, :])
                    m2e = EP.tile([128, D], F32, tag="m2e")
                    nc.scalar.activation(m2e[:cw, :], mu_t[:cw, :], AF.Square,
                                         scale=a3s_c)
                    v1 = EP.tile([128, D], F32, tag="v1")
                    v2 = EP.tile([128, D], F32, tag="v2")
                    nc.vector.tensor_scalar_mul(v1[:cw, :], wq_ps[:cw, :], r_c)
                    nc.scalar.activation(v2[:cw, :], var_t[:cw, :], AF.Copy, scale=a1_c)
                    nc.gpsimd.tensor_tensor(out=v1[:cw, :], in0=v1[:cw, :],
                                            in1=v2[:cw, :], op=OP.add)
                    nc.vector.tensor_tensor(out=q[:cw, :], in0=q[:cw, :],
                                            in1=m2e[:cw, :], op=OP.add)
                    varo_t = EP.tile([128, D], F32, tag="varo_t")
                    nc.vector.tensor_tensor(out=varo_t[:cw, :], in0=v1[:cw, :],
                                            in1=q[:cw, :], op=OP.add)
                    nc.vector.tensor_scalar_max(varo_t[:cw, :], varo_t[:cw, :], VARMIN)
                    nc.sync.dma_start(varo_d[ksl, :], varo_t[:cw, :])

    nc.compile()
    return nc


_NC = None


def _get_nc():
    global _NC
    if _NC is None:
        _NC = build_nc()
    return _NC


def make_in_maps(x, gamma_class, mu, var, pi, c):
    x = np.ascontiguousarray(np.asarray(x, np.float32))
    gamma_class = np.ascontiguousarray(np.asarray(gamma_class, np.float32))
    mu = np.asarray(mu, np.float32)
    var = np.asarray(var, np.float32)
    pi = np.asarray(pi, np.float32)
    c = np.asarray(c, np.float32)
    xT = np.ascontiguousarray(x.T)
    in_maps = []
    for i in range(NCORES):
        ks, ke = i * KC, (i + 1) * KC
        mu_i = np.ascontiguousarray(mu[ks:ke].reshape(KM, D))
        var_i = np.ascontiguousarray(var[ks:ke].reshape(KM, D))
        in_maps.append({
            "xT": xT,
            "x": x,
            "varT": np.ascontiguousarray(var_i.T),
            "muT": np.ascontiguousarray(mu_i.T),
            "var": var_i,
            "mu": mu_i,
            "gc": np.ascontiguousarray(gamma_class[:, ks:ke]),
            "c": np.ascontiguousarray(c[ks:ke].reshape(1, KM)),
            "pi": np.ascontiguousarray(pi[ks:ke].reshape(1, KM)),
        })
    return in_maps


def kernel(x, gamma_class, mu, var, pi, c):
    in_maps = make_in_maps(x, gamma_class, mu, var, pi, c)
    nc = _get_nc()
    out = run_bass_kernel_spmd(nc, in_maps, core_ids=list(range(NCORES)))
    rs = out.results
    mu_new = np.concatenate([rs[i]["mu_new"].reshape(KC, M, D) for i in range(NCORES)], 0)
    var_new = np.concatenate([rs[i]["var_new"].reshape(KC, M, D) for i in range(NCORES)], 0)
    c_new = np.concatenate([rs[i]["c_new"].reshape(KC, M) for i in range(NCORES)], 0)
    pi_new = np.concatenate([rs[i]["pi_new"].reshape(KC, M) for i in range(NCORES)], 0)
    return (np.asarray(mu_new, np.float32), np.asarray(var_new, np.float32),
            np.asarray(c_new, np.float32), np.asarray(pi_new, np.float32))


# revision 17
# speedup vs baseline: 1.3263x; 1.0973x over previous
"""DOTA_mix.fit() streaming EM update on Trainium2 — 8-core K-sharded SPMD.

kernel(x, gamma_class, mu, var, pi, c) -> (mu_new, var_new, c_new, pi_new)

Strategy (per core, classes sharded 1000 -> 8 x 125):
  J[b,km] = log pi + log N(x | mu,var) is computed as ONE fused matmul chain
    J = x2 @ W1c + x @ W2c + constc      (contraction dim 512+512+1)
  where W1c/W2c/constc are per-class mean-centered over the 4 modes, which
  makes mean_m J == 0 exactly, so softmax over modes needs NO max pass
  (centering is softmax-invariant; an extra -40 shift bounds exp above).
  gamma = gamma_class * softmax_m(J);  then one more matmul batch:
    [weighted_x | weighted_x_sq] = gamma^T @ [x | x*x],  sum_gamma = 1^T gamma
  followed by the cheap EM update algebra.
Matmuls run in fp32r (TRN2 fast-fp32, ~13-bit mantissa) at full rate.
"""
import numpy as np
import concourse.bass as bass
import concourse.bacc as bacc
import concourse.tile as tile
from concourse import mybir
from concourse.masks import make_identity
from concourse.bass_utils import run_bass_kernel_spmd

F32 = mybir.dt.float32
F32R = mybir.dt.float32r
AF = mybir.ActivationFunctionType
OP = mybir.AluOpType
AX = mybir.AxisListType

B, K, M, D = 2048, 1000, 4, 512
NCORES = 8
KC = K // NCORES          # 125 classes per core
KM = KC * M               # 500 (class,mode) rows per core
NBT = B // 128            # 16 batch tiles
EPS, TINY, VARMIN = 1e-3, 1e-10, 1e-8
SHIFT = 40.0
CH = [(0, 128), (128, 128), (256, 128), (384, 116)]  # km chunks


def build_nc():
    nc = bacc.Bacc("TRN2", target_bir_lowering=False, debug=False)

    xT_d = nc.dram_tensor("xT", [D, B], F32, kind="ExternalInput")
    x_d = nc.dram_tensor("x", [B, D], F32, kind="ExternalInput")
    varT_d = nc.dram_tensor("varT", [D, KM], F32, kind="ExternalInput")
    muT_d = nc.dram_tensor("muT", [D, KM], F32, kind="ExternalInput")
    var_d = nc.dram_tensor("var", [KM, D], F32, kind="ExternalInput")
    mu_d = nc.dram_tensor("mu", [KM, D], F32, kind="ExternalInput")
    gc_d = nc.dram_tensor("gc", [B, KC], F32, kind="ExternalInput")
    c_d = nc.dram_tensor("c", [1, KM], F32, kind="ExternalInput")
    pi_d = nc.dram_tensor("pi", [1, KM], F32, kind="ExternalInput")

    muo_d = nc.dram_tensor("mu_new", [KM, D], F32, kind="ExternalOutput")
    varo_d = nc.dram_tensor("var_new", [KM, D], F32, kind="ExternalOutput")
    co_d = nc.dram_tensor("c_new", [1, KM], F32, kind="ExternalOutput")
    pio_d = nc.dram_tensor("pi_new", [1, KM], F32, kind="ExternalOutput")

    with tile.TileContext(nc) as tc:
        import contextlib
        with contextlib.ExitStack() as ctx:
            A = ctx.enter_context(tc.tile_pool(name="A", bufs=1))

            # ---- persistent tiles -------------------------------------------------
            gam_sb = A.tile([128, NBT, KM], F32R, tag="gam")    # responsibilities
            ones_col = A.tile([128, 1], F32, tag="ones_col")
            ones_colr = A.tile([128, 1], F32R, tag="ones_colr")
            ones_row = A.tile([1, 128], F32, tag="ones_row")
            ones_rowr = A.tile([1, 128], F32R, tag="ones_rowr")
            constr = A.tile([1, KM], F32R, tag="constr")
            c_sb = A.tile([1, KM], F32, tag="c_sb")
            pi_sb = A.tile([1, KM], F32, tag="pi_sb")

            nc.sync.dma_start(c_sb[:], c_d[:])
            nc.sync.dma_start(pi_sb[:], pi_d[:])
            nc.vector.memset(ones_col[:], 1.0)
            nc.vector.tensor_copy(ones_colr[:], ones_col[:])
            nc.vector.memset(ones_row[:], 1.0)
            nc.vector.tensor_copy(ones_rowr[:], ones_row[:])

            psSG = ctx.enter_context(tc.tile_pool(name="psSG", bufs=1, space="PSUM"))
            sg_ps = None

            with tc.tile_pool(name="BP", bufs=1) as BP:
                xtr_sb = BP.tile([128, 4, B], F32R, tag="xtr")    # d-major x, f32r
                xxT_sb = BP.tile([128, 4, B], F32R, tag="xxT")    # (x*x)^T rounded
                W1r = BP.tile([128, 4, KM], F32R, tag="W1r")      # centered -0.5/var, d-major
                W2r = BP.tile([128, 4, KM], F32R, tag="W2r")      # centered mu/var, d-major

                # ---- phase 0: load + square + round x/xT; W prep -----------------
                with tc.tile_pool(name="CP", bufs=1) as CP, \
                     tc.tile_pool(name="psR", bufs=1, space="PSUM") as psR:
                    for i in range(4):
                        xTf = CP.tile([128, B], F32, tag="xTf", bufs=2)
                        nc.sync.dma_start(
                            xTf[:], xT_d[:].rearrange("(c p) b -> p c b", p=128)[:, i, :])
                        nc.scalar.activation(xxT_sb[:, i, :], xTf[:], AF.Square)
                        nc.scalar.copy(xtr_sb[:, i, :], xTf[:])
                    ld_ps = psR.tile([1, 512], F32, tag="ld")
                    m2_ps = psR.tile([1, 512], F32, tag="m2")

                    for i in range(4):
                        varT = CP.tile([128, KM], F32, tag="varT", bufs=2)
                        muT = CP.tile([128, KM], F32, tag="muT", bufs=2)
                        nc.sync.dma_start(
                            varT[:], varT_d[:].rearrange("(c p) k -> p c k", p=128)[:, i, :])
                        nc.sync.dma_start(
                            muT[:], muT_d[:].rearrange("(c p) k -> p c k", p=128)[:, i, :])
                        inv = CP.tile([128, KM], F32, tag="inv")
                        lg = CP.tile([128, KM], F32, tag="lgw")
                        gmean = CP.tile([128, KC], F32, tag="gmean")

                        cv = varT[:]
                        nc.vector.tensor_scalar_add(cv, cv, EPS)   # in place
                        nc.vector.reciprocal_approx_fast(inv[:], cv)
                        nc.scalar.activation(lg[:], cv, AF.Ln)
                        # reduction over d via ones-matmul (full fp32 for accuracy)
                        nc.tensor.matmul(ld_ps[:, :KM], ones_col[:], lg[:],
                                         start=(i == 0), stop=(i == 3))
                        # center W1 = -0.5*inv over modes:  W1c = -0.5*inv + 0.125*gsum(inv)
                        nc.vector.reduce_sum(out=gmean[:],
                                             in_=inv[:].rearrange("p (k m) -> p k m", m=4),
                                             axis=AX.X)
                        nc.vector.tensor_scalar_mul(gmean[:], gmean[:], 0.125)
                        nc.vector.tensor_scalar_mul(
                            W1r[:, i, :].rearrange("p (k m) -> p k m", m=4),
                            inv[:].rearrange("p (k m) -> p k m", m=4), -0.5)
                        nc.vector.tensor_tensor(
                            out=W1r[:, i, :].rearrange("p (k m) -> p k m", m=4),
                            in0=W1r[:, i, :].rearrange("p (k m) -> p k m", m=4),
                            in1=gmean[:].unsqueeze(-1).broadcast_to((128, KC, 4)),
                            op=OP.add)
                        # w2f reuses lg's slot (after the ld matmul consumed lg)
                        w2f = CP.tile([128, KM], F32, tag="lgw")
                        nc.gpsimd.tensor_mul(w2f[:], muT[:], inv[:])
                        # center W2 = mu*inv over modes
                        nc.vector.reduce_sum(out=gmean[:],
                                             in_=w2f[:].rearrange("p (k m) -> p k m", m=4),
                                             axis=AX.X)
                        nc.vector.tensor_scalar_mul(gmean[:], gmean[:], 0.25)
                        nc.vector.tensor_tensor(
                            out=W2r[:, i, :].rearrange("p (k m) -> p k m", m=4),
                            in0=w2f[:].rearrange("p (k m) -> p k m", m=4),
                            in1=gmean[:].unsqueeze(-1).broadcast_to((128, KC, 4)),
                            op=OP.subtract)
                        # mwi = mu^2/var reuses inv's slot (inv fully consumed)
                        mwi = CP.tile([128, KM], F32, tag="inv")
                        nc.gpsimd.tensor_mul(mwi[:], muT[:], w2f[:])
                        nc.tensor.matmul(m2_ps[:, :KM], ones_col[:], mwi[:],
                                         start=(i == 0), stop=(i == 3))

                    # const row: log(pi+TINY) - 0.5*(log_det+m2), centered, -SHIFT
                    t_row = CP.tile([1, KM], F32, tag="t_row")
                    pl_row = CP.tile([1, KM], F32, tag="pl_row")
                    g_row = CP.tile([1, KC], F32, tag="g_row")
                    nc.vector.tensor_scalar_add(pl_row[:], pi_sb[:], TINY)
                    nc.scalar.activation(pl_row[:], pl_row[:], AF.Ln)
                    nc.scalar.copy(t_row[:], m2_ps[:, :KM])
                    nc.vector.tensor_tensor(out=t_row[:], in0=ld_ps[:, :KM],
                                            in1=t_row[:], op=OP.add)
                    nc.vector.tensor_scalar(out=t_row[:], in0=t_row[:],
                                            scalar1=-0.5, scalar2=None, op0=OP.mult)
                    nc.vector.tensor_tensor(out=t_row[:], in0=t_row[:], in1=pl_row[:],
                                            op=OP.add)
                    nc.vector.reduce_sum(out=g_row[:],
                                         in_=t_row[:].rearrange("p (k m) -> p k m", m=4),
                                         axis=AX.X)
                    nc.vector.tensor_scalar(out=g_row[:], in0=g_row[:], scalar1=0.25,
                                            scalar2=SHIFT, op0=OP.mult, op1=OP.add)
                    nc.vector.tensor_tensor(
                        out=constr[:].rearrange("p (k m) -> p k m", m=4),
                        in0=t_row[:].rearrange("p (k m) -> p k m", m=4),
                        in1=g_row[:].unsqueeze(-1).broadcast_to((1, KC, 4)),
                        op=OP.subtract)

                # ---- phase 1: J matmul + modewise softmax + gamma ----------------
                sg_ps = psSG.tile([1, 512], F32, tag="sg")
                with tc.tile_pool(name="GP", bufs=1) as GP, \
                     tc.tile_pool(name="DP", bufs=3) as DP, \
                     tc.tile_pool(name="psJ", bufs=3, space="PSUM") as psJ:
                    gc_sb = GP.tile([128, NBT, KC], F32, tag="gc")
                    nc.sync.dma_start(gc_sb[:], gc_d[:].rearrange("(n p) k -> p n k", p=128))
                    for bt in range(NBT):
                        bsl = slice(bt * 128, (bt + 1) * 128)
                        J_ps = psJ.tile([128, 512], F32, tag="J")
                        for i in range(4):
                            nc.tensor.matmul(J_ps[:, :KM], xxT_sb[:, i, bsl],
                                             W1r[:, i, :], start=(i == 0), stop=False)
                        for i in range(4):
                            nc.tensor.matmul(J_ps[:, :KM],
                                             xtr_sb[:, i, bsl],
                                             W2r[:, i, :], start=False, stop=False)
                        nc.tensor.matmul(J_ps[:, :KM], ones_rowr[:], constr[:],
                                         start=False, stop=True)
                        E = DP.tile([128, KM], F32, tag="E")
                        nc.scalar.activation(E[:], J_ps[:, :KM], AF.Exp)
                        s4 = DP.tile([128, KC], F32, tag="s4")
                        nc.vector.reduce_sum(out=s4[:],
                                             in_=E[:].rearrange("p (k m) -> p k m", m=4),
                                             axis=AX.X)
                        wgt = DP.tile([128, KC], F32, tag="wgt")
                        nc.vector.reciprocal_approx_fast(wgt[:], s4[:])
                        nc.vector.tensor_mul(wgt[:], wgt[:], gc_sb[:, bt, :])
                        nc.gpsimd.tensor_tensor(
                            out=gam_sb[:, bt, :].rearrange("p (k m) -> p k m", m=4),
                            in0=E[:].rearrange("p (k m) -> p k m", m=4),
                            in1=wgt[:].unsqueeze(-1).broadcast_to((128, KC, 4)),
                            op=OP.mult)
                        nc.tensor.matmul(sg_ps[:, :KM], ones_colr[:], gam_sb[:, bt, :],
                                         start=(bt == 0), stop=(bt == NBT - 1))

            # ---- phase 2a: per-km scalars (row layout), c_new, pi_new ------------
            F2 = ctx.enter_context(tc.tile_pool(name="F2", bufs=1))
            cn_row = F2.tile([1, KM], F32, tag="cn_row")
            pin_row = F2.tile([1, KM], F32, tag="pin_row")
            scals = F2.tile([5, KM], F32, tag="scals")           # r, a1, a2, a3, sqrt(a3) rows
            cols = F2.tile([128, 4, 5], F32, tag="cols")         # transposed scalars / km chunk
            pig = F2.tile([1, KC], F32, tag="pig")
            ident4 = F2.tile([5, 5], F32, tag="ident4")
            make_identity(nc, ident4[:])

            r_row = F2.tile([1, KM], F32, tag="r_row")
            a_row = F2.tile([1, KM], F32, tag="a_row")
            nc.vector.tensor_tensor(out=cn_row[:], in0=c_sb[:], in1=sg_ps[:, :KM],
                                    op=OP.add)
            nc.sync.dma_start(co_d[:], cn_row[:])
            # r = 1/(c_new + TINY)
            nc.vector.tensor_scalar_add(r_row[:], cn_row[:], TINY)
            nc.vector.reciprocal_approx_fast(r_row[:], r_row[:])
            nc.sync.dma_start(scals[0:1, :], r_row[:])
            # a1 = c * r
            nc.vector.tensor_tensor(out=a_row[:], in0=c_sb[:], in1=r_row[:],
                                    op=OP.mult)
            nc.sync.dma_start(scals[1:2, :], a_row[:])
            # a2 = -2r
            a2_row = F2.tile([1, KM], F32, tag="a2_row")
            nc.vector.tensor_scalar_mul(a2_row[:], r_row[:], -2.0)
            nc.sync.dma_start(scals[2:3, :], a2_row[:])
            # a3 = sum_gamma * r
            a3_row = F2.tile([1, KM], F32, tag="a3_row")
            nc.vector.tensor_tensor(out=a3_row[:], in0=sg_ps[:, :KM],
                                    in1=r_row[:], op=OP.mult)
            nc.sync.dma_start(scals[3:4, :], a3_row[:])
            a3s_row = F2.tile([1, KM], F32, tag="a3s_row")
            nc.scalar.activation(a3s_row[:], a3_row[:], AF.Sqrt)
            nc.sync.dma_start(scals[4:5, :], a3s_row[:])
            # pi_new = c_new / (gsum_m c_new + TINY)
            nc.vector.reduce_sum(out=pig[:],
                                 in_=cn_row[:].rearrange("p (k m) -> p k m", m=4),
                                 axis=AX.X)
            nc.vector.tensor_scalar_add(pig[:], pig[:], TINY)
            nc.vector.reciprocal_approx_fast(pig[:], pig[:])
            nc.vector.tensor_tensor(
                out=pin_row[:].rearrange("p (k m) -> p k m", m=4),
                in0=cn_row[:].rearrange("p (k m) -> p k m", m=4),
                in1=pig[:].unsqueeze(-1).broadcast_to((1, KC, 4)), op=OP.mult)
            nc.sync.dma_start(pio_d[:], pin_row[:])

            # transpose scalar rows into per-chunk columns
            with tc.tile_pool(name="psT", bufs=2, space="PSUM") as psT:
                for j, (j0, cw) in enumerate(CH):
                    t_ps = psT.tile([128, 5], F32, tag="tps")
                    nc.tensor.transpose(t_ps[:cw, :], scals[:, j0:j0 + cw],
                                        ident4[:])
                    nc.scalar.copy(cols[:cw, j, :], t_ps[:cw, :])

            # ---- phase 2b: gamma^T @ [x|xx] and EM update ------------------------
            with tc.tile_pool(name="EP", bufs=2) as EP, \
                 tc.tile_pool(name="psW", bufs=2, space="PSUM") as psW:
                xr_sb = EP.tile([128, NBT, D], F32R, tag="xr", bufs=1)
                xx_sb = EP.tile([128, NBT, D], F32R, tag="xx", bufs=1)
                for i in range(4):
                    xf = EP.tile([128, 4, D], F32, tag="xf")
                    nc.sync.dma_start(
                        xf[:], x_d[:].rearrange("(n p) d -> p n d", p=128)[:, 4 * i:4 * i + 4, :])
                    nc.scalar.activation(xx_sb[:, 4 * i:4 * i + 4, :], xf[:], AF.Square)
                    nc.vector.tensor_copy(xr_sb[:, 4 * i:4 * i + 4, :], xf[:])
                for j, (j0, cw) in enumerate(CH):
                    ksl = slice(j0, j0 + cw)
                    wx_ps = psW.tile([128, 512], F32, tag="wx")
                    wq_ps = psW.tile([128, 512], F32, tag="wq")
                    for bt in range(NBT):
                        nc.tensor.matmul(wx_ps[:cw, :], gam_sb[:, bt, ksl],
                                         xr_sb[:, bt, :],
                                         start=(bt == 0), stop=(bt == NBT - 1))
                    for bt in range(NBT):
                        nc.tensor.matmul(wq_ps[:cw, :], gam_sb[:, bt, ksl],
                                         xx_sb[:, bt, :],
                                         start=(bt == 0), stop=(bt == NBT - 1))

                    r_c = cols[:cw, j, 0:1]
                    a1_c = cols[:cw, j, 1:2]
                    a2_c = cols[:cw, j, 2:3]
                    a3s_c = cols[:cw, j, 4:5]

                    mu_t = EP.tile([128, D], F32, tag="mu_t")
                    var_t = EP.tile([128, D], F32, tag="var_t")
                    nc.sync.dma_start(mu_t[:cw, :], mu_d[ksl, :])
                    nc.sync.dma_start(var_t[:cw, :], var_d[ksl, :])

                    # mu_new = mu*a1 + wx*r
                    u1 = EP.tile([128, D], F32, tag="u1")
                    u2 = EP.tile([128, D], F32, tag="u2")
                    nc.vector.tensor_scalar_mul(u1[:cw, :], wx_ps[:cw, :], r_c)
                    nc.scalar.activation(u2[:cw, :], mu_t[:cw, :], AF.Copy, scale=a1_c)
                    muo_t = EP.tile([128, D], F32, tag="muo_t")
                    nc.gpsimd.tensor_tensor(out=muo_t[:cw, :], in0=u1[:cw, :],
                                            in1=u2[:cw, :], op=OP.add)
                    nc.sync.dma_start(muo_d[ksl, :], muo_t[:cw, :])

                    # var_new = max(wxsq*r + var*a1 + (mu*a2)*wx + (mu*sqrt(a3))^2, VARMIN)
                    ma2 = EP.tile([128, D], F32, tag="ma2")
                    nc.scalar.activation(ma2[:cw, :], mu_t[:cw, :], AF.Copy, scale=a2_c)
                    q = EP.tile([128, D], F32, tag="q")
                    nc.vector.tensor_mul(q[:cw, :], ma2[:cw, :], wx_ps[:cw, :])
                    m2e = EP.tile([128, D], F32, tag="m2e")
                    nc.scalar.activation(m2e[:cw, :], mu_t[:cw, :], AF.Square,
                                         scale=a3s_c)
                    v1 = EP.tile([128, D], F32, tag="v1")
                    v2 = EP.tile([128, D], F32, tag="v2")
                    nc.vector.tensor_scalar_mul(v1[:cw, :], wq_ps[:cw, :], r_c)
                    nc.scalar.activation(v2[:cw, :], var_t[:cw, :], AF.Copy, scale=a1_c)
                    nc.gpsimd.tensor_tensor(out=v1[:cw, :], in0=v1[:cw, :],
                                            in1=v2[:cw, :], op=OP.add)
                    nc.vector.tensor_tensor(out=q[:cw, :], in0=q[:cw, :],
                                            in1=m2e[:cw, :], op=OP.add)
                    varo_t = EP.tile([128, D], F32, tag="varo_t")
                    nc.vector.tensor_tensor(out=varo_t[:cw, :], in0=v1[:cw, :],
                                            in1=q[:cw, :], op=OP.add)
                    nc.vector.tensor_scalar_max(varo_t[:cw, :], varo_t[:cw, :], VARMIN)
                    nc.sync.dma_start(varo_d[ksl, :], varo_t[:cw, :])

    nc.compile()
    return nc


_NC = None


def _get_nc():
    global _NC
    if _NC is None:
        _NC = build_nc()
    return _NC


def make_in_maps(x, gamma_class, mu, var, pi, c):
    x = np.ascontiguousarray(np.asarray(x, np.float32))
    gamma_class = np.ascontiguousarray(np.asarray(gamma_class, np.float32))
    mu = np.asarray(mu, np.float32)
    var = np.asarray(var, np.float32)
    pi = np.asarray(pi, np.float32)
    c = np.asarray(c, np.float32)
    xT = np.ascontiguousarray(x.T)
    in_maps = []
    for i in range(NCORES):
        ks, ke = i * KC, (i + 1) * KC
        mu_i = np.ascontiguousarray(mu[ks:ke].reshape(KM, D))
        var_i = np.ascontiguousarray(var[ks:ke].reshape(KM, D))
        in_maps.append({
            "xT": xT,
            "x": x,
            "varT": np.ascontiguousarray(var_i.T),
            "muT": np.ascontiguousarray(mu_i.T),
            "var": var_i,
            "mu": mu_i,
            "gc": np.ascontiguousarray(gamma_class[:, ks:ke]),
            "c": np.ascontiguousarray(c[ks:ke].reshape(1, KM)),
            "pi": np.ascontiguousarray(pi[ks:ke].reshape(1, KM)),
        })
    return in_maps


def kernel(x, gamma_class, mu, var, pi, c):
    in_maps = make_in_maps(x, gamma_class, mu, var, pi, c)
    nc = _get_nc()
    out = run_bass_kernel_spmd(nc, in_maps, core_ids=list(range(NCORES)))
    rs = out.results
    mu_new = np.concatenate([rs[i]["mu_new"].reshape(KC, M, D) for i in range(NCORES)], 0)
    var_new = np.concatenate([rs[i]["var_new"].reshape(KC, M, D) for i in range(NCORES)], 0)
    c_new = np.concatenate([rs[i]["c_new"].reshape(KC, M) for i in range(NCORES)], 0)
    pi_new = np.concatenate([rs[i]["pi_new"].reshape(KC, M) for i in range(NCORES)], 0)
    return (np.asarray(mu_new, np.float32), np.asarray(var_new, np.float32),
            np.asarray(c_new, np.float32), np.asarray(pi_new, np.float32))
